# revision 5
# baseline (speedup 1.0000x reference)
"""BuildCostVolume kernel for 8 Trainium2 NeuronCores (round-robin rewrite).

Decomposition as kernel.py: 729 taps (d,u,v), each a K=64(ci) x M=64(co)
matmul over spatial positions, 4-way concurrent via tile_position
(2 row-groups x 2 col-groups).

Changes vs kernel.py:
 - MM emission is position-ROUND-ROBIN (one sub-MM per position visit)
   instead of 2-3 back-to-back same-position MMs per tap.  Same-position
   MMs serialize (pc-monotone starts + same-subarray), so the old pass-1
   ran ~2-way; round-robin approaches true 4-way.
 - Row clipping: out rows h with h + d*(4-u) < 0 read zero pad rows; the
   MMs now clip them from the AP (rlo per sub) instead of multiplying
   zeros (~9% of all columns).  The 4 seed taps stay unclipped so their
   start=True writes cover the full psum region (they read the memset pad
   rows of tiles 0/1 -- the only memsets left).
 - Class-flip host packing makes the clipping class-independent: h-half-1
   cores get view-relabeled (u,v)->(8-u,8-v), h/w-flipped view images and
   a kernel-flipped weight table (W[:, :, ::-1, ::-1]).  Device program is
   IDENTICAL for all cores; out is unflipped host-side.  (Derivation: with
   Y[ci,u0,v0,r,c] = X[ci,8-u0,8-v0,47-r,47-c] and flipped W-table, the
   class-1 half satisfies out[co,d,47-h',47-w'] = dev_out[co,d,h',w'].)
   This also removes the tc.If(core-id) branch in the input load.
 - PSUM as 8 explicit bank tags (bufs=1) rotated FIFO across walks so a
   new walk lands on the banks whose evacs were emitted earliest.
 - sub-2 walks pair same-sign disparities (-4,-3), (-2,-1), (1,2), (3,4):
   same kidx per tap => one LDW per 2 MMs (dedup'd by _dedup_ldweights).
"""

import os
from collections import deque

import numpy as np
import ml_dtypes

A = 9           # angular resolution
H = 48          # spatial h/w per view
C = 64          # channels (ci = co = 64)
B = 4           # batch
ND = 9          # disparities -4..4
HH = 24         # h rows per core (half)
SUB = 8         # output h rows per psum accumulation group
NSUB = HH // SUB
N_CORES = 8

BF16 = ml_dtypes.bfloat16

POS_ORDER = ((0, 0), (1, 0), (0, 1), (1, 1))


def _geometry():
    """Static tap/tile geometry shared by host packing and device program."""
    pairs = []            # (viewA, viewB-or-None, R)
    # seed tiles first: their 4 views are the full-width (v=4) taps that
    # carry start=True per tile position, so their DMAs must land first.
    pairs.append(((0, 4), (8, 4), HH + 8 * 4))
    pairs.append(((1, 4), (7, 4), HH + 8 * 3))
    for v in range(A):
        for u in range(4):
            if v == 4 and u in (0, 1):
                continue
            pairs.append(((u, v), (8 - u, v), HH + 8 * (4 - u)))
    for k in range(4):
        pairs.append(((4, k), (4, k + 5), HH))
    pairs.append(((4, 4), None, HH))
    # alternate which view of a pair sits on which partition half: row
    # clipping at fixed d hits only u<4 (or only u>4) views, so keeping
    # all u<4 on half 0 would starve one row-group at large |d|.
    pairs = [(vb, va, R) if (j % 2 == 1 and vb is not None) else (va, vb, R)
             for j, (va, vb, R) in enumerate(pairs)]

    view_loc = {}
    offs = []
    off = 0
    for j, (va, vb, R) in enumerate(pairs):
        view_loc[va] = (j, 0)
        if vb is not None:
            view_loc[vb] = (j, 1)
        offs.append(off)
        off += R * H
    F = off

    taps = []  # (u, v, rh) in tile order; ch assigned per walk
    for j, (va, vb, R) in enumerate(pairs):
        for half, view in ((0, va), (1, vb)):
            if view is None:
                continue
            taps.append((view[0], view[1], half))
    assert len(taps) == 81
    assert all(v == 4 for (u, v, _) in taps[:4])
    assert {rh for (_, _, rh) in taps[:4]} == {0, 1}
    seeds = {(u, v) for (u, v, _) in taps[:4]}

    return pairs, view_loc, offs, F, taps, seeds


_PAIRS, _VIEW_LOC, _OFFS, _F, _TAPS, _SEEDS = _geometry()
_NC_CACHE = {}


def _rlo(d, u, v, sub):
    """Rows clipped from the top of this sub's 8-row window (class-0
    geometry; out row h needs x row h + d*(4-u) >= 0).  Seeds unclipped."""
    if (u, v) in _SEEDS:
        return 0
    return max(0, min(SUB, -d * (4 - u) - sub * SUB))


def _walk_cols(tap, dsubs):
    """Exact column count this tap contributes to its quadrant in a walk
    covering dsubs.  The (4,4) singleton is counted per-rh (half-width)."""
    u, v, rh = tap
    c = 0
    for (d, sub) in dsubs:
        if (u, v) == (4, 4):
            c += SUB * (H // 2)
            continue
        r = _rlo(d, u, v, sub)
        if r < SUB:
            c += (SUB - r) * (H - abs(d * (4 - v)))
    return c


def _assign_ch(dsubs):
    """Per-walk greedy column-half assignment balancing quadrant columns
    within each row-half.  Seeds forced to opposite ch so every position's
    queue starts with a full-extent start=True tap.  Returns
    {(u,v): ch} plus ("44", rh) entries for the singleton's half-MMs."""
    load = {p: 0 for p in POS_ORDER}
    chmap = {}
    for rh in (0, 1):
        rtaps = [t for t in _TAPS if t[2] == rh and (t[0], t[1]) != (4, 4)]
        sd = [t for t in rtaps if (t[0], t[1]) in _SEEDS]
        for ch, t in enumerate(sd):
            chmap[(t[0], t[1])] = ch
            load[(rh, ch)] += _walk_cols(t, dsubs)
        rest = sorted([t for t in rtaps if (t[0], t[1]) not in _SEEDS],
                      key=lambda t: -_walk_cols(t, dsubs))
        for t in rest:
            ch = 0 if load[(rh, 0)] <= load[(rh, 1)] else 1
            chmap[(t[0], t[1])] = ch
            load[(rh, ch)] += _walk_cols(t, dsubs)
    # (4,4) singleton: one half-width MM per rh, each on the lighter ch
    c44 = _walk_cols((4, 4, 0), dsubs)
    for rh in (0, 1):
        ch = 0 if load[(rh, 0)] <= load[(rh, 1)] else 1
        chmap[("44", rh)] = ch
        load[(rh, ch)] += c44
    return chmap


def _build_nc(repeat=1):
    import concourse.bacc as bacc
    import concourse.mybir as mybir
    import concourse.tile as tile

    variant = os.environ.get("KVARIANT", "full")  # timing experiments only

    nc = bacc.Bacc(None, target_bir_lowering=False)
    xwin_d = nc.dram_tensor("xwin", [128, _F], mybir.dt.bfloat16,
                            kind="ExternalInput")
    wt_d = nc.dram_tensor("wt", [128, A * A * C], mybir.dt.bfloat16,
                          kind="ExternalInput")
    out_d = nc.dram_tensor("out", [C, ND * NSUB * SUB * H], mybir.dt.float32,
                           kind="ExternalOutput")

    with tile.TileContext(nc) as tc:
        with tc.tile_pool(name="xw", bufs=1) as xpool, \
             tc.tile_pool(name="wp", bufs=1) as wpool, \
             tc.tile_pool(name="ps", bufs=1, space="PSUM") as ppool, \
             tc.tile_pool(name="ob", bufs=4) as opool:

            # resident weight taps; DMA first so the seed MMs aren't blocked
            wtr = wpool.tile([128, A * A * C], mybir.dt.bfloat16, tag="wtr")
            nc.sync.dma_start(out=wtr[:], in_=wt_d[:])

            xtiles = []
            xviews = []
            for j, (va, vb, R) in enumerate(_PAIRS):
                t = xpool.tile([128, R * H], mybir.dt.bfloat16, tag=f"x{j}")
                xtiles.append(t)
                xviews.append(t[:].rearrange("p (r c) -> p r c", r=R, c=H))

            def load_x():
                # Uniform class-0 geometry for every core (class-flip host
                # packing): view block rows [0, 4au) are pad, valid image
                # rows land at [4au, R).  Only the two SEED tiles' pad rows
                # are ever read (other taps are row-clipped), so only those
                # get DVE memsets.
                for j, (va, vb, R) in enumerate(_PAIRS):
                    pad = ((R - HH) // 8) * 4  # 4*au
                    if pad and j < 2:
                        nc.vector.memset(xtiles[j][:, 0:pad * H], 0.0)
                    nc.sync.dma_start(
                        out=xtiles[j][:, pad * H:R * H],
                        in_=xwin_d[:, _OFFS[j] + pad * H:_OFFS[j] + R * H])

            pool_tags = deque([f"pb{i}" for i in range(8)])

            def alloc_tiles(dsubs):
                tiles = {}
                order = []
                for (d, sub) in dsubs:
                    for rh in (0, 1):
                        tag = pool_tags.popleft()
                        tiles[(d, sub, rh)] = ppool.tile(
                            [128, SUB * H], mybir.dt.float32,
                            name=tag, tag=tag)
                        order.append(tag)
                return tiles, order

            def mm(d, sub, tap, ch, tiles, started):
                (u, v, rh) = tap
                j, half = _VIEW_LOC[(u, v)]
                au = abs(4 - u)
                sft = d * (4 - u)
                row0 = sub * SUB + sft + 4 * au
                sv = d * (4 - v)
                wlo = max(0, -sv)
                whi = min(H, H - sv)
                rlo = _rlo(d, u, v, sub)
                if rlo >= SUB:
                    return
                if variant == "halfcols":
                    # timing-only: halve every MM's width (same instruction
                    # count) to separate issue-rate-bound from datapath-bound
                    whi = wlo + max(1, (whi - wlo) // 2)
                elif variant == "fullw2":
                    # timing-only: force full-extent flat 2D APs on both
                    # sides (wrong numerics; +22% columns) to quantify the
                    # cost of clipped/3D access patterns
                    sv = 0
                    wlo, whi, rlo = 0, H, 0
                rhs = xviews[j][rh * 64:(rh + 1) * 64,
                                row0 + rlo:row0 + SUB,
                                wlo + sv:whi + sv]
                kh, kw = (u, v) if d <= 0 else (8 - u, 8 - v)
                kidx = kh * A + kw
                lhsT = wtr[rh * 64:(rh + 1) * 64,
                           kidx * C:(kidx + 1) * C]
                pt = tiles[(d, sub, rh)]
                key = (d, sub, rh, ch)
                if rlo == 0 and wlo == 0 and whi == H:
                    outap = pt[ch * 64:(ch + 1) * 64, :]
                else:
                    ptv = pt[:].rearrange("p (r c) -> p r c", r=SUB, c=H)
                    outap = ptv[ch * 64:(ch + 1) * 64, rlo:SUB, wlo:whi]
                nc.tensor.matmul(
                    outap, lhsT, rhs,
                    start=(key not in started),
                    stop=False,
                    tile_position=(rh * 64, ch * 64),
                    skip_group_check=True,
                )
                started.add(key)

            def mm44(d, sub, chmap, tiles, started):
                # (4,4) singleton: no shift/clip for any d; split into two
                # half-width MMs, one per row-half (data duplicated on both
                # tile halves), each on that rh's lightest quadrant.
                j, _ = _VIEW_LOC[(4, 4)]
                kidx = 4 * A + 4
                row0 = sub * SUB
                for rh, c0, c1 in ((0, 0, H // 2), (1, H // 2, H)):
                    ch = chmap[("44", rh)]
                    rhs = xviews[j][rh * 64:(rh + 1) * 64,
                                    row0:row0 + SUB, c0:c1]
                    lhsT = wtr[rh * 64:(rh + 1) * 64,
                               kidx * C:(kidx + 1) * C]
                    pt = tiles[(d, sub, rh)]
                    ptv = pt[:].rearrange("p (r c) -> p r c", r=SUB, c=H)
                    outap = ptv[ch * 64:(ch + 1) * 64, 0:SUB, c0:c1]
                    key = (d, sub, rh, ch)
                    nc.tensor.matmul(
                        outap, lhsT, rhs,
                        start=(key not in started), stop=False,
                        tile_position=(rh * 64, ch * 64),
                        skip_group_check=True,
                    )
                    started.add(key)

            def emit1(d, sub, tap, chmap, tiles, started):
                if (tap[0], tap[1]) == (4, 4):
                    mm44(d, sub, chmap, tiles, started)
                else:
                    mm(d, sub, tap, chmap[(tap[0], tap[1])], tiles, started)

            def _variant_chmap(chmap):
                # timing-only: force every tap onto ch0 (or ch1) positions
                if variant in ("ch0only", "ch1only"):
                    f = 0 if variant == "ch0only" else 1
                    return {k: f for k in chmap}
                return chmap

            def emit_walk(dsubs, tiles, started):
                # 4-position round-robin, one sub-MM per visit; each tap's
                # dsubs are consecutive in its position queue (same lhsT =>
                # LDWs dedup to one per tap).
                chmap = _variant_chmap(_assign_ch(dsubs))
                posq = {p: [] for p in POS_ORDER}
                for t in _TAPS:
                    key = ("44", 0) if (t[0], t[1]) == (4, 4) else (t[0], t[1])
                    posq[(t[2], chmap[key])].append(t)
                # Duration-sort each queue (seed pinned first): the strict
                # round-robin issue is gated by each round's slowest MM, so
                # rounds should carry like-sized MMs.  E[max of 4 mixed
                # widths] ~ 1.15x mean costs ~15% of the 4-way rate.
                for p in POS_ORDER:
                    sd = [t for t in posq[p] if (t[0], t[1]) in _SEEDS]
                    rest = sorted(
                        [t for t in posq[p] if (t[0], t[1]) not in _SEEDS],
                        key=lambda t: -_walk_cols(t, dsubs))
                    posq[p] = sd + rest
                queues = [[(t, d, s) for t in posq[p] for (d, s) in dsubs]
                          for p in POS_ORDER]
                while any(queues):
                    for q in queues:
                        if q:
                            t, d, s = q.pop(0)
                            emit1(d, s, t, chmap, tiles, started)

            def emit_load_walk(dsubs, tiles, started):
                # Tile-arrival-order emission: interleave consecutive PAIRS
                # of x-tiles so the PE runs ~4-way without waiting on
                # undelivered tiles.  ch is assigned per GROUP so each
                # 2-pair group covers all 4 positions, with the running
                # quadrant load deciding which tap gets which ch.
                groups = [list(range(g, min(g + 2, len(_PAIRS))))
                          for g in range(0, len(_PAIRS), 2)]
                ti = 0
                tap_of_pair = []
                for j, (va, vb, R) in enumerate(_PAIRS):
                    n = 1 if vb is None else 2
                    tap_of_pair.append(_TAPS[ti:ti + n])
                    ti += n
                load = {p: 0 for p in POS_ORDER}
                chmap = {}
                for grp in groups:
                    gtaps = [t for j in grp for t in tap_of_pair[j]]
                    for rh in (0, 1):
                        rtaps = sorted(
                            [t for t in gtaps
                             if t[2] == rh and (t[0], t[1]) != (4, 4)],
                            key=lambda t: -_walk_cols(t, dsubs))
                        for t in rtaps:
                            ch = 0 if load[(rh, 0)] <= load[(rh, 1)] else 1
                            if (t[0], t[1]) in chmap:
                                ch = chmap[(t[0], t[1])]
                            else:
                                chmap[(t[0], t[1])] = ch
                            load[(rh, ch)] += _walk_cols(t, dsubs)
                            # force the group's 2nd same-rh tap to the
                            # other ch so the group covers 4 positions
                            load[(rh, 1 - ch)] += 0
                        if len(rtaps) == 2:
                            a, b = rtaps
                            if chmap[(a[0], a[1])] == chmap[(b[0], b[1])]:
                                old = chmap[(b[0], b[1])]
                                chmap[(b[0], b[1])] = 1 - old
                                load[(rh, old)] -= _walk_cols(b, dsubs)
                                load[(rh, 1 - old)] += _walk_cols(b, dsubs)
                    for rh in (0, 1):
                        if ("44", rh) not in chmap and any(
                                (t[0], t[1]) == (4, 4) for t in gtaps):
                            chmap[("44", rh)] = (
                                0 if load[(rh, 0)] <= load[(rh, 1)] else 1)
                    chmap = _variant_chmap(chmap)
                    for (d, s) in dsubs:
                        for t in gtaps:
                            emit1(d, s, t, chmap, tiles, started)

            def evac(di, sub, ptA, ptB):
                ot = opool.tile([64, SUB * H], mybir.dt.float32, tag="ot")
                # Cross-partition reads only from PSUM, one PSUM operand/op.
                if variant == "tinyadd":
                    # timing-only: same dep structure, 1/48th the DVE data
                    nc.scalar.activation(ot[:], ptA[0:64, :],
                                         mybir.ActivationFunctionType.Copy)
                    nc.vector.tensor_add(ot[:, 0:8], ot[:, 0:8],
                                         ptA[64:128, 0:8])
                    nc.vector.tensor_add(ot[:, 0:8], ot[:, 0:8],
                                         ptB[0:64, 0:8])
                    nc.vector.tensor_add(ot[:, 0:8], ot[:, 0:8],
                                         ptB[64:128, 0:8])
                elif variant == "chainadd":
                    # previous structure: Act copy + 3 serial DVE adds
                    nc.scalar.activation(ot[:], ptA[0:64, :],
                                         mybir.ActivationFunctionType.Copy)
                    nc.vector.tensor_add(ot[:], ot[:], ptA[64:128, :])
                    nc.vector.tensor_add(ot[:], ot[:], ptB[0:64, :])
                    nc.vector.tensor_add(ot[:], ot[:], ptB[64:128, :])
                else:
                    # tree evac: two Act copies drain one quadrant-half of
                    # each bank, DVE adds the other two (one PSUM operand
                    # per op), and the SBUF-SBUF combine goes to whichever
                    # engine the scheduler picks (Pool is idle) -- DVE does
                    # 2 psum ops instead of 3.
                    ot2 = opool.tile([64, SUB * H], mybir.dt.float32,
                                     tag="ot2")
                    nc.scalar.activation(ot[:], ptA[0:64, :],
                                         mybir.ActivationFunctionType.Copy)
                    nc.scalar.activation(ot2[:], ptB[0:64, :],
                                         mybir.ActivationFunctionType.Copy)
                    nc.vector.tensor_add(ot[:], ot[:], ptA[64:128, :])
                    nc.vector.tensor_add(ot2[:], ot2[:], ptB[64:128, :])
                    # SBUF-SBUF combine on the otherwise-idle Pool engine
                    # (nc.any leaves it on DVE; Pool cannot read PSUM but
                    # this op is pure SBUF)
                    nc.engines[mybir.EngineType.Pool].tensor_add(
                        ot[:], ot[:], ot2[:])
                seg = (di * NSUB + sub) * SUB * H
                nc.sync.dma_start(out=out_d[:, seg:seg + SUB * H], in_=ot[:])

            def emit_evacs(dsubs, tiles, order):
                for (d, sub) in dsubs:
                    evac(d + 4, sub, tiles[(d, sub, 0)], tiles[(d, sub, 1)])
                pool_tags.extend(order)

            def emit_sweep():
                started = set()
                # load walk: d=-4,-3 subs 0,1 (8 banks), overlapping the
                # HBM-bound input stream in tile-arrival order
                ds_load = [(-4, 0), (-4, 1), (-3, 0), (-3, 1)]
                tl, ol = alloc_tiles(ds_load)
                emit_load_walk(ds_load, tl, started)
                emit_evacs(ds_load, tl, ol)
                # paired sub-2 walk for d=-4,-3 (same kidx => shared LDW)
                ds = [(-4, 2), (-3, 2)]
                tb, ob = alloc_tiles(ds)
                emit_walk(ds, tb, started)
                emit_evacs(ds, tb, ob)
                for dpair in ((-2, -1), (0,), (1, 2), (3, 4)):
                    for d in dpair:
                        ds = [(d, 0), (d, 1)]
                        t_, o_ = alloc_tiles(ds)
                        emit_walk(ds, t_, started)
                        emit_evacs(ds, t_, o_)
                    ds = [(d, 2) for d in dpair]
                    t_, o_ = alloc_tiles(ds)
                    emit_walk(ds, t_, started)
                    emit_evacs(ds, t_, o_)

            if repeat == 1:
                load_x()
                emit_sweep()
            elif variant == "loopall":
                # timing: input DMA repeats with the sweep => loop slope
                # approximates a full single-shot exec
                with tc.For_i(0, repeat, 1):
                    load_x()
                    emit_sweep()
            else:
                # timing: repeat the compute sweep in a hardware loop
                load_x()
                with tc.For_i(0, repeat, 1):
                    emit_sweep()

    _dedup_ldweights(nc, drop_all=(variant == "noldw"))
    if variant != "nocollapse":
        _collapse_pe_sem_incs(nc)
    nc.finalize()
    return nc


def _collapse_pe_sem_incs(nc):
    """Collapse per-MM semaphore increments into one bulk sem-inc at the
    end of each run of WAIT-FREE PE instructions.  Per-MM then_incs
    serialize on the EVT_SEM register (~26ns each, tensor-engine doc) --
    one inc per MM caps the sweep at ~26ns/MM regardless of tile-position
    concurrency.  Moving incs LATER is always data-safe (thresholds clear
    later, never earlier); deadlock is impossible because a run contains
    only wait-free PE instructions, so the PE always reaches the run end
    where the bulk inc fires.  Runs break at any PE instruction that
    waits (walk-first MMs, psum-WAR guards, LDW-vs-MM guards) and at any
    PE instruction that isn't a plain matmul/ldweights."""
    removed = 0
    for fn in nc.m.functions:
        for bb in fn.blocks:
            cur = []  # (instruction, update) collected since last break

            def flush(run):
                nonlocal removed
                by_sem = {}
                for ins, upd in run:
                    by_sem.setdefault(upd.id, []).append((ins, upd))
                for sem, lst in by_sem.items():
                    if len(lst) < 2:
                        continue
                    total = sum(u.update_value for _, u in lst)
                    # 'sem-inc' adds 1 regardless of update_value; the bulk
                    # increment needs the add-immediate form (same mode the
                    # SWDGE DMA completion updates use).
                    lst[-1][1].update_mode = "sem-add-imm"
                    lst[-1][1].update_value = total
                    for ins, upd in lst[:-1]:
                        ins.sync_info.on_update.remove(upd)
                        removed += 1

            for ins in bb.instructions:
                if not str(ins.engine).endswith("PE"):
                    continue
                tn = type(ins).__name__
                si = ins.sync_info
                has_wait = si is not None and bool(si.on_wait)
                if tn not in ("InstMatmult", "InstLdweights"):
                    flush(cur)
                    cur = []
                    continue
                if has_wait:
                    flush(cur)
                    cur = []
                if si and si.on_update:
                    for upd in list(si.on_update):
                        if (upd.sync_type == "semaphore"
                                and upd.update_mode == "sem-inc"):
                            cur.append((ins, upd))
            flush(cur)
    if removed:
        import logging
        logging.getLogger(__name__).info(
            "collapse_pe_sem_incs: removed %d increments", removed)


def _dedup_ldweights(nc, drop_all=False):
    """Remove InstLdweights that reload the stationary operand already
    resident at the same tile position.  The PE keeps independent
    stationary sets per (row, col) tile group, and only an LDW targeting
    the same position clobbers one.  drop_all (timing-only variant):
    remove every sync-free LDW after the first per position."""
    removed = kept = 0
    for bb in nc.m.functions[0].blocks:
        last = {}
        to_remove = []
        for ins in bb.instructions:
            if not str(ins.engine).endswith("PE"):
                continue
            tn = type(ins).__name__
            if tn == "InstLdweights":
                si = ins.sync_info
                has_sync = si is not None and (si.on_wait or si.on_update)
                sig = (str(ins.ins[0]), str(getattr(ins, "tile_position", None)),
                       str(getattr(ins, "perf_mode", None)))
                pos = str(getattr(ins, "tile_position", None))
                if drop_all:
                    sig = True
                if not has_sync and last.get(pos) == sig:
                    to_remove.append(ins)
                    removed += 1
                else:
                    last[pos] = sig
                    kept += 1
            elif tn == "InstMatmult":
                continue
            else:
                last.clear()
        for ins in to_remove:
            bb.instructions.remove(ins)
    if removed:
        import logging
        logging.getLogger(__name__).info(
            "dedup_ldweights: removed %d, kept %d", removed, kept)


def get_nc(repeat=1):
    key = ("nc", repeat, os.environ.get("KVARIANT", "full"))
    if key not in _NC_CACHE:
        _NC_CACHE[key] = _build_nc(repeat)
    return _NC_CACHE[key]


def prepare_inputs(x, W):
    """Host-side packing: per-core xwin [128,F] bf16 + per-class weights.

    h-half-1 cores get the class-flip: view (u0,v0) holds
    X[8-u0, 8-v0, ::-1(h), ::-1(w)] and the weight table is kernel-flipped,
    which makes the device program identical to the h-half-0 one."""
    x = np.asarray(x, dtype=np.float32)
    W = np.asarray(W, dtype=np.float32)
    # X5[b,u,v,ci,h,w]
    X5 = np.ascontiguousarray(
        x.reshape(B, C, H, A, H, A).transpose(0, 3, 5, 1, 2, 4)
    ).astype(BF16)

    xwins = []
    for core in range(N_CORES):
        b, hh = divmod(core, 2)
        V = X5[b] if hh == 0 else X5[b, ::-1, ::-1, :, ::-1, ::-1]
        xw = np.zeros((128, _F), dtype=BF16)
        for j, (va, vb, R) in enumerate(_PAIRS):
            # the (4,4) singleton is duplicated onto the (otherwise empty)
            # second half of its tile so its matmul can be split across
            # both row-halves for quadrant load balance.
            for half, view in ((0, va), (1, vb if vb is not None else va)):
                u, v = view
                lo = -4 * abs(4 - u)
                ve = min(H, lo + R)
                blk = V[u, v, :, 0:ve, :]  # [64, ve, 48]
                dst = xw[half * 64:(half + 1) * 64,
                         _OFFS[j]:_OFFS[j] + R * H].reshape(64, R, H)
                dst[:, -lo:ve - lo, :] = blk
        xwins.append(xw)

    # wt[ci + 64*half, (kh*9+kw)*64 + co] = Wc[co, ci, kh, kw], where Wc is
    # W for h-half-0 cores and W[:, :, ::-1, ::-1] for h-half-1 cores.
    def pack_w(Wc):
        w1 = np.ascontiguousarray(
            Wc.transpose(1, 2, 3, 0).reshape(C, A * A * C)).astype(BF16)
        return np.concatenate([w1, w1], axis=0)

    return xwins, (pack_w(W), pack_w(W[:, :, ::-1, ::-1]))


def assemble_output(results):
    """results: list of 8 dicts with 'out' [64, ND*NSUB*SUB*H] fp32."""
    full = np.empty((B, C, ND, H, H), dtype=np.float32)
    for core in range(N_CORES):
        b, hh = divmod(core, 2)
        oc = np.asarray(results[core]["out"]).reshape(C, ND, HH, H)
        if hh == 0:
            full[b, :, :, 0:HH, :] = oc
        else:
            full[b, :, :, HH:H, :] = oc[:, :, ::-1, ::-1]
    return full


def make_in_maps(x, W):
    xwins, (wt0, wt1) = prepare_inputs(x, W)
    return [{"xwin": xwins[c], "wt": (wt0 if c % 2 == 0 else wt1)}
            for c in range(N_CORES)]


def kernel(x, W):
    from concourse.bass_utils import run_bass_kernel_spmd

    nc = get_nc()
    in_maps = make_in_maps(x, W)
    res = run_bass_kernel_spmd(nc, in_maps, core_ids=list(range(N_CORES)))
    return assemble_output(res.results)
